# revision 11
# baseline (speedup 1.0000x reference)
"""MoEMixer Trainium2 Bass kernel (nn_MoEMixer_66949950210414).

Data-parallel over batch across 8 NeuronCores (4 samples/core).
Per sample: gate (masked mean-pool -> LN -> MLP -> top-2 softmax) in fp32;
per selected expert, weights are fetched from the stacked [E, ...] DRAM
tensors via register-offset dynamic DMA (cast to bf16) and the expert block
runs in transposed-activation layout [D, S]:
  hn    = LN(x)*lg+lb          (stats via natural-layout DVE reduces,
                                applied transposed with PE row-broadcasts)
  convT = depthwise conv in S  (5 PSUM-accumulating matmuls with diagonal
                                bf16 weight matrices)
  base  = convT + cb ; y = x + base
  yn    = LN(y)*lg+lb          (stats via ones-vector bf16 matmuls)
  ffT   = gelu(W1^T yn + b1); o2T = W2^T ffT + b2   (bf16 matmuls; stacked
          weights are the stationary operand -> no weight transposes)
  acc  += wk * (base + o2T)
Because the top-2 weights sum to 1, out = mask^2 * (x + acc); the exact fp32
x residual and the mask are applied at finalize in natural layout after the
PE transpose back.
"""
import numpy as np

import concourse.bacc as bacc
import concourse.tile as tile
from concourse import mybir
from concourse.bass import ds
from concourse.bass_utils import run_bass_kernel_spmd

F32 = mybir.dt.float32
BF16 = mybir.dt.bfloat16
U32 = mybir.dt.uint32
AF = mybir.ActivationFunctionType
OP = mybir.AluOpType
AX = mybir.AxisListType

B, S, D, E, DFF, KS = 32, 1024, 512, 8, 2048, 5
NCORES = 8
NSAMP = B // NCORES          # 4
SC = S // 128                # 8
DC = D // 128                # 4
KC = DFF // 128              # 16
NH = 2
H = S // NH                  # 512
PAD = KS // 2                # 2
EPS = 1e-5


def build_program(gelu_func=AF.Gelu):
    nc = bacc.Bacc(None, target_bir_lowering=False)

    x_ext = nc.declare_dram_parameter("x_sh", [NSAMP * S, D], F32, isOutput=False)
    mask_ext = nc.declare_dram_parameter("mask_sh", [NSAMP, S], F32, isOutput=False)
    ln_g_ext = nc.declare_dram_parameter("ln_g", [E, D], F32, isOutput=False)
    ln_b_ext = nc.declare_dram_parameter("ln_b", [E, D], F32, isOutput=False)
    cw_ext = nc.declare_dram_parameter("conv_w", [E, D, KS], F32, isOutput=False)
    cb_ext = nc.declare_dram_parameter("conv_b", [E, D], F32, isOutput=False)
    w1_ext = nc.declare_dram_parameter("w1", [E, D, DFF], F32, isOutput=False)
    b1_ext = nc.declare_dram_parameter("b1", [E, DFF], F32, isOutput=False)
    w2_ext = nc.declare_dram_parameter("w2", [E, DFF, D], F32, isOutput=False)
    b2_ext = nc.declare_dram_parameter("b2", [E, D], F32, isOutput=False)
    gln_g_ext = nc.declare_dram_parameter("gln_g", [1, D], F32, isOutput=False)
    gln_b_ext = nc.declare_dram_parameter("gln_b", [1, D], F32, isOutput=False)
    gw1_ext = nc.declare_dram_parameter("gw1", [D, D], F32, isOutput=False)
    gb1_ext = nc.declare_dram_parameter("gb1", [1, D], F32, isOutput=False)
    gw2_ext = nc.declare_dram_parameter("gw2", [D, E], F32, isOutput=False)
    gb2_ext = nc.declare_dram_parameter("gb2", [1, E], F32, isOutput=False)
    out_ext = nc.declare_dram_parameter("out", [NSAMP * S, D], F32, isOutput=True)

    with tile.TileContext(nc) as tc:
        ctxs = []

        def pool(name, bufs, space=None):
            kw = {"space": space} if space else {}
            p = tc.tile_pool(name=name, bufs=bufs, **kw)
            ctxs.append(p)
            return p.__enter__()

        const = pool("const", 1)
        xin = pool("xin", 2)          # streamed natural x chunks
        samp = pool("samp", 2)        # per-sample persistent (xTb, xhatT, mask cols)
        acc_pool = pool("acc", 1)
        w1p = pool("w1p", 1)
        w2p = pool("w2p", 1)
        slotp = pool("slotp", 2)
        ffgp = pool("ffgp", 3)
        rows = pool("rows", 1)
        cols = pool("cols", 4)
        onat_p = pool("onat", 2)
        psum_po = pool("psum_po", 4, "PSUM")
        psum_wk = pool("psum_wk", 2, "PSUM")
        psum_st = pool("psum_st", 2, "PSUM")

        # ---------------- constants ----------------
        ident = const.tile([128, 128], F32)
        from concourse.masks import make_identity
        make_identity(nc, ident[:])
        ident_bf = const.tile([128, 128], BF16)
        nc.vector.tensor_copy(ident_bf[:], ident[:])

        ones_col_bf = const.tile([128, 1], BF16)
        nc.gpsimd.memset(ones_col_bf[:], 1.0)
        ones_row_bf = const.tile([1, 128], BF16)
        nc.gpsimd.memset(ones_row_bf[:], 1.0)

        gw1_t = const.tile([128, DC, D], F32)
        nc.sync.dma_start(gw1_t[:], gw1_ext[:, :].rearrange("(c p) f -> p c f", p=128))
        gw2_t = const.tile([128, DC, E], F32)
        nc.sync.dma_start(gw2_t[:], gw2_ext[:, :].rearrange("(c p) f -> p c f", p=128))
        gln_g_row = const.tile([1, D], F32)
        nc.sync.dma_start(gln_g_row[:], gln_g_ext[:, :])
        gln_b_row = const.tile([1, D], F32)
        nc.sync.dma_start(gln_b_row[:], gln_b_ext[:, :])
        gb1_row = const.tile([1, D], F32)
        nc.sync.dma_start(gb1_row[:], gb1_ext[:, :])
        gb2_row = const.tile([1, E], F32)
        nc.sync.dma_start(gb2_row[:], gb2_ext[:, :])

        st = [dict() for _ in range(NSAMP)]

        def bcast(row_bf, h_or_none=None):
            """[1, W] bf16 row -> [128, W] bf16 tile via K=1 matmul."""
            W = row_bf.shape[-1]
            pb = psum_wk.tile([128, W], F32, tag="work", name="pb")
            nc.tensor.matmul(pb[:], ones_row_bf[:], row_bf, start=True, stop=True)
            return pb

        def phase_a(b):
            d = st[b]
            # mask pieces
            mask_row = rows.tile([1, S], F32, tag="mask_row")
            nc.sync.dma_start(mask_row[:], mask_ext[b:b + 1, :])
            mask_cols = samp.tile([128, SC], F32, tag="mask_cols")
            for sc in range(SC):
                nc.sync.dma_start(
                    mask_cols[:, sc:sc + 1],
                    mask_ext[b:b + 1, sc * 128:(sc + 1) * 128].rearrange("o p -> p o"),
                )
            msq_col = samp.tile([128, SC], F32, tag="msq_col")
            nc.vector.tensor_tensor(msq_col[:], mask_cols[:], mask_cols[:], OP.mult)
            d["msq_col"] = msq_col

            xTb = samp.tile([128, DC, S], BF16, tag="xTb")
            sx_c = rows.tile([128, SC], F32, tag="sx_c")
            sq_c = rows.tile([128, SC], F32, tag="sq_c")
            p_gn = psum_st.tile([1, D], F32, tag="stat")
            for sc in range(SC):
                x_sc = xin.tile([128, D], F32, tag="x_sc")
                nc.sync.dma_start(x_sc[:], x_ext[b * S + sc * 128:b * S + (sc + 1) * 128, :])
                # pooled gate input: sum_s x*m
                nc.tensor.matmul(p_gn[:], mask_cols[:, sc:sc + 1], x_sc[:],
                                 start=(sc == 0), stop=(sc == SC - 1))
                # per-position sums for LN1 stats
                nc.vector.tensor_reduce(sx_c[:, sc:sc + 1], x_sc[:], AX.X, OP.add)
                xsq = xin.tile([128, D], BF16, tag="xsq")
                nc.scalar.activation(xsq[:], x_sc[:], AF.Square)
                nc.vector.tensor_reduce(sq_c[:, sc:sc + 1], xsq[:], AX.X, OP.add)
                # transpose x chunk -> bf16 xT
                for dc in range(DC):
                    pt = psum_wk.tile([128, 128], F32, tag="work")
                    nc.tensor.transpose(pt[:], x_sc[:, dc * 128:(dc + 1) * 128], ident[:])
                    nc.vector.tensor_copy(xTb[:, dc, sc * 128:(sc + 1) * 128], pt[:])
            d["xTb"] = xTb

            # gate pooled vector
            dsum = rows.tile([1, 1], F32, tag="dsum")
            nc.vector.tensor_reduce(dsum[:], mask_row[:], AX.X, OP.add)
            nc.vector.tensor_scalar(dsum[:], dsum[:], 1.0, None, OP.max)
            dinv = rows.tile([1, 1], F32, tag="dinv")
            nc.vector.reciprocal(dinv[:], dsum[:])
            g_row = rows.tile([1, D], F32, tag="g_row")
            nc.vector.tensor_scalar(g_row[:], p_gn[:], dinv[0:1, 0:1], None, OP.mult)
            d["g_row"] = g_row

            # LN1 rows: m1, rstd1 per position (row scratch reused in place)
            m1_row = rows.tile([1, S], F32, tag="m1_row")
            q1_row = rows.tile([1, S], F32, tag="q1_row")
            for (src, dst) in ((sx_c, m1_row), (sq_c, q1_row)):
                for sc in range(SC):
                    pt = psum_wk.tile([128, 128], F32, tag="work")
                    nc.tensor.transpose(pt[0:1, :], src[:, sc:sc + 1], ident[:])
                    nc.vector.tensor_scalar(dst[:, sc * 128:(sc + 1) * 128],
                                            pt[0:1, :], 1.0 / D, None, OP.mult)
            v1_row = rows.tile([1, S], F32, tag="v1_row")
            nc.vector.tensor_tensor(v1_row[:], m1_row[:], m1_row[:], OP.mult)
            nc.vector.tensor_tensor(v1_row[:], q1_row[:], v1_row[:], OP.subtract)
            nc.vector.tensor_scalar(v1_row[:], v1_row[:], EPS, None, OP.add)
            nc.scalar.activation(v1_row[:], v1_row[:], AF.Sqrt)
            nc.vector.reciprocal(v1_row[:], v1_row[:])          # = rstd1 (fp32)
            rstd1_bf = rows.tile([1, S], BF16, tag="rstd1_bf")
            nc.vector.tensor_copy(rstd1_bf[:], v1_row[:])
            mr1_bf = rows.tile([1, S], BF16, tag="mr1_bf")
            nc.vector.tensor_tensor(mr1_bf[:], m1_row[:], v1_row[:], OP.mult)

            # xhatT = (x - m1) * rstd1   (bf16; conv input only)
            xhatT = samp.tile([128, DC, S], BF16, tag="xhatT")
            for h in range(NH):
                hs = slice(h * H, (h + 1) * H)
                r_b = slotp.tile([128, H], BF16, tag="r1_b")
                m_b = slotp.tile([128, H], BF16, tag="m1_b")
                for (row, bct) in ((rstd1_bf, r_b), (mr1_bf, m_b)):
                    pb = psum_wk.tile([128, H], F32, tag="work")
                    nc.tensor.matmul(pb[:], ones_row_bf[:], row[:, hs], start=True, stop=True)
                    nc.scalar.activation(bct[:], pb[:], AF.Copy)
                for dc in range(DC):
                    nc.vector.tensor_tensor(xhatT[:, dc, hs], xTb[:, dc, hs], r_b[:], OP.mult)
                    nc.vector.tensor_tensor(xhatT[:, dc, hs], xhatT[:, dc, hs], m_b[:], OP.subtract)
            d["xhatT"] = xhatT

        def gate(b):
            d = st[b]
            g_row = d["g_row"]
            mg = rows.tile([1, 1], F32, tag="mg")
            nc.vector.tensor_reduce(mg[:], g_row[:], AX.X, OP.add)
            nc.vector.tensor_scalar(mg[:], mg[:], 1.0 / D, None, OP.mult)
            gt = rows.tile([1, D], F32, tag="gt")
            nc.vector.tensor_scalar(gt[:], g_row[:], mg[0:1, 0:1], None, OP.subtract)
            gsq = rows.tile([1, D], F32, tag="gsq")
            nc.scalar.activation(gsq[:], gt[:], AF.Square)
            vg = rows.tile([1, 1], F32, tag="vg")
            nc.vector.tensor_reduce(vg[:], gsq[:], AX.X, OP.add)
            nc.vector.tensor_scalar(vg[:], vg[:], 1.0 / D, None, OP.mult)
            nc.vector.tensor_scalar(vg[:], vg[:], EPS, None, OP.add)
            nc.scalar.activation(vg[:], vg[:], AF.Sqrt)
            rg = rows.tile([1, 1], F32, tag="rg")
            nc.vector.reciprocal(rg[:], vg[:])
            h_row = rows.tile([1, D], F32, tag="h_row")
            nc.vector.tensor_scalar(h_row[:], gt[:], rg[0:1, 0:1], None, OP.mult)
            nc.vector.tensor_tensor(h_row[:], h_row[:], gln_g_row[:], OP.mult)
            nc.vector.tensor_tensor(h_row[:], h_row[:], gln_b_row[:], OP.add)
            # h @ gw1 + gb1 -> gelu
            hT = cols.tile([128, DC], F32, tag="hT")
            for kc in range(DC):
                pt = psum_wk.tile([128, 128], F32, tag="work")
                nc.tensor.transpose(pt[:, 0:1], h_row[0:1, kc * 128:(kc + 1) * 128],
                                    ident[0:1, 0:1])
                nc.vector.tensor_copy(hT[:, kc:kc + 1], pt[:, 0:1])
            p_h1 = psum_st.tile([1, D], F32, tag="stat")
            for kc in range(DC):
                nc.tensor.matmul(p_h1[:], hT[:, kc:kc + 1], gw1_t[:, kc, :],
                                 start=(kc == 0), stop=(kc == DC - 1))
            h1_row = rows.tile([1, D], F32, tag="h1_row")
            nc.vector.tensor_tensor(h1_row[:], p_h1[:], gb1_row[:], OP.add)
            hg_row = rows.tile([1, D], F32, tag="hg_row")
            nc.scalar.activation(hg_row[:], h1_row[:], gelu_func)
            # logits
            hgT = cols.tile([128, DC], F32, tag="hgT")
            for kc in range(DC):
                pt = psum_wk.tile([128, 128], F32, tag="work")
                nc.tensor.transpose(pt[:, 0:1], hg_row[0:1, kc * 128:(kc + 1) * 128],
                                    ident[0:1, 0:1])
                nc.vector.tensor_copy(hgT[:, kc:kc + 1], pt[:, 0:1])
            p_lg = psum_st.tile([1, E], F32, tag="stat")
            for kc in range(DC):
                nc.tensor.matmul(p_lg[:], hgT[:, kc:kc + 1], gw2_t[:, kc, :],
                                 start=(kc == 0), stop=(kc == DC - 1))
            logits = rows.tile([1, E], F32, tag="logits")
            nc.vector.tensor_tensor(logits[:], p_lg[:], gb2_row[:], OP.add)
            vals8 = rows.tile([1, 8], F32, tag="vals8")
            idx8 = rows.tile([1, 8], U32, tag="idx8", bufs=2)
            nc.vector.max_with_indices(vals8[:], idx8[:], logits[:])
            d["idx8"] = idx8
            # softmax over top-2
            dd = rows.tile([1, 1], F32, tag="dd")
            nc.vector.tensor_scalar(dd[:], vals8[:, 1:2], vals8[0:1, 0:1], None, OP.subtract)
            ee = rows.tile([1, 1], F32, tag="ee")
            nc.scalar.activation(ee[:], dd[:], AF.Exp)
            nc.vector.tensor_scalar(ee[:], ee[:], 1.0, None, OP.add)
            wA = rows.tile([1, 1], F32, tag="wA")
            nc.vector.reciprocal(wA[:], ee[:])
            wB = rows.tile([1, 1], F32, tag="wB")
            nc.vector.tensor_scalar(wB[:], wA[:], -1.0, 1.0, OP.mult, OP.add)
            wcols = cols.tile([128, 2], F32, tag="wcols")
            nc.gpsimd.partition_broadcast(wcols[:, 0:1], wA[:])
            nc.gpsimd.partition_broadcast(wcols[:, 1:2], wB[:])
            d["wcols"] = wcols

        def slot(b, k):
            d = st[b]
            idx8 = d["idx8"]
            ei_sp = nc.sync.value_load(idx8[0:1, k:k + 1])
            ei_gp = nc.gpsimd.value_load(idx8[0:1, k:k + 1])

            # dynamic weight loads (fp32 -> bf16 cast on SWDGE)
            w1_t = w1p.tile([128, DC, DFF], BF16, tag="w1_t")
            nc.gpsimd.dma_start(
                w1_t[:], w1_ext[ds(ei_gp, 1), :, :].rearrange("o (c p) f -> (o p) c f", p=128))
            w2_t = w2p.tile([128, KC, D], BF16, tag="w2_t")
            nc.gpsimd.dma_start(
                w2_t[:], w2_ext[ds(ei_gp, 1), :, :].rearrange("o (c p) f -> (o p) c f", p=128))
            cw_t = cols.tile([128, DC, KS], F32, tag="cw_t")
            nc.sync.dma_start(
                cw_t[:], cw_ext[ds(ei_sp, 1), :, :].rearrange("o (c p) j -> (o p) c j", p=128))
            lg_c = cols.tile([128, DC], F32, tag="lg_c")
            lb_c = cols.tile([128, DC], F32, tag="lb_c")
            cb_c = cols.tile([128, DC], F32, tag="cb_c")
            b2_c = cols.tile([128, DC], F32, tag="b2_c")
            for dc in range(DC):
                csl = slice(dc * 128, (dc + 1) * 128)
                nc.sync.dma_start(lg_c[:, dc:dc + 1], ln_g_ext[ds(ei_sp, 1), csl].rearrange("o p -> p o"))
                nc.sync.dma_start(lb_c[:, dc:dc + 1], ln_b_ext[ds(ei_sp, 1), csl].rearrange("o p -> p o"))
                nc.sync.dma_start(cb_c[:, dc:dc + 1], cb_ext[ds(ei_sp, 1), csl].rearrange("o p -> p o"))
                nc.sync.dma_start(b2_c[:, dc:dc + 1], b2_ext[ds(ei_sp, 1), csl].rearrange("o p -> p o"))
            b1_c = cols.tile([128, KC], F32, tag="b1_c")
            for kc in range(KC):
                nc.sync.dma_start(
                    b1_c[:, kc:kc + 1],
                    b1_ext[ds(ei_sp, 1), kc * 128:(kc + 1) * 128].rearrange("o p -> p o"))

            xhatT = d["xhatT"]
            xTb = d["xTb"]
            wk_col = d["wcols"][:, k:k + 1]
            if k == 0:
                d["out_acc"] = acc_pool.tile([128, DC, S], BF16, name="out_acc", tag="out_acc")
            out_acc = d["out_acc"]

            for h in range(NH):
                hs = slice(h * H, (h + 1) * H)
                # hn (padded by PAD cols each side, halves overlap via xhatT)
                hn_h = slotp.tile([128, DC, H + 2 * PAD], BF16, tag="hn_h")
                for dc in range(DC):
                    if h == 0:
                        nc.gpsimd.memset(hn_h[:, dc, 0:PAD], 0.0)
                        nc.vector.tensor_scalar(hn_h[:, dc, PAD:H + 2 * PAD],
                                                xhatT[:, dc, 0:H + PAD],
                                                lg_c[:, dc:dc + 1], lb_c[:, dc:dc + 1],
                                                OP.mult, OP.add)
                    else:
                        nc.gpsimd.memset(hn_h[:, dc, H + PAD:H + 2 * PAD], 0.0)
                        nc.vector.tensor_scalar(hn_h[:, dc, 0:H + PAD],
                                                xhatT[:, dc, h * H - PAD:S],
                                                lg_c[:, dc:dc + 1], lb_c[:, dc:dc + 1],
                                                OP.mult, OP.add)
                # conv -> base (conv + cb); y_st = base + x for LN2 stats
                base = slotp.tile([128, DC, H], BF16, tag="base")
                p_sy = psum_st.tile([1, H], F32, tag="stat")
                p_sq = psum_st.tile([1, H], F32, tag="stat")
                for dc in range(DC):
                    diag = slotp.tile([128, KS, 128], BF16, tag="diag")
                    for j in range(KS):
                        nc.vector.tensor_scalar(diag[:, j, :], ident_bf[:],
                                                cw_t[:, dc, j:j + 1], None, OP.mult)
                    p_y = psum_wk.tile([128, H], F32, tag="work")
                    for j in range(KS):
                        nc.tensor.matmul(p_y[:], diag[:, j, :], hn_h[:, dc, j:j + H],
                                         start=(j == 0), stop=(j == KS - 1))
                    nc.vector.tensor_scalar(base[:, dc, :], p_y[:],
                                            cb_c[:, dc:dc + 1], None, OP.add)
                    y_st = slotp.tile([128, H], BF16, tag="y_st")
                    nc.vector.tensor_tensor(y_st[:], base[:, dc, :], xTb[:, dc, hs], OP.add)
                    ysq = slotp.tile([128, H], BF16, tag="ysq")
                    nc.scalar.activation(ysq[:], y_st[:], AF.Square)
                    nc.tensor.matmul(p_sy[:], ones_col_bf[:], y_st[:],
                                     start=(dc == 0), stop=(dc == DC - 1))
                    nc.tensor.matmul(p_sq[:], ones_col_bf[:], ysq[:],
                                     start=(dc == 0), stop=(dc == DC - 1))
                # LN2 rows (scratch reused in place)
                m2_row = rows.tile([1, H], F32, tag="m2_row")
                nc.vector.tensor_scalar(m2_row[:], p_sy[:], 1.0 / D, None, OP.mult)
                v2_row = rows.tile([1, H], F32, tag="v2_row")
                nc.vector.tensor_tensor(v2_row[:], m2_row[:], m2_row[:], OP.mult)
                q2_row = rows.tile([1, H], F32, tag="q2_row")
                nc.vector.tensor_scalar(q2_row[:], p_sq[:], 1.0 / D, None, OP.mult)
                nc.vector.tensor_tensor(v2_row[:], q2_row[:], v2_row[:], OP.subtract)
                nc.vector.tensor_scalar(v2_row[:], v2_row[:], EPS, None, OP.add)
                nc.scalar.activation(v2_row[:], v2_row[:], AF.Sqrt)
                nc.vector.reciprocal(v2_row[:], v2_row[:])       # = rstd2
                rstd2_bf = rows.tile([1, H], BF16, tag="rstd2_bf")
                nc.vector.tensor_copy(rstd2_bf[:], v2_row[:])
                mr2_bf = rows.tile([1, H], BF16, tag="mr2_bf")
                nc.vector.tensor_tensor(mr2_bf[:], m2_row[:], v2_row[:], OP.mult)
                rstd2_b = slotp.tile([128, H], BF16, tag="rstd2_b")
                mr2_b = slotp.tile([128, H], BF16, tag="mr2_b")
                for (row, bct) in ((rstd2_bf, rstd2_b), (mr2_bf, mr2_b)):
                    pb = psum_wk.tile([128, H], F32, tag="work")
                    nc.tensor.matmul(pb[:], ones_row_bf[:], row[:], start=True, stop=True)
                    nc.scalar.activation(bct[:], pb[:], AF.Copy)
                # ynT (bf16)
                ynT = slotp.tile([128, DC, H], BF16, tag="ynT")
                for dc in range(DC):
                    t1 = slotp.tile([128, H], BF16, tag="y_st")
                    nc.vector.tensor_tensor(t1[:], base[:, dc, :], xTb[:, dc, hs], OP.add)
                    nc.vector.tensor_tensor(t1[:], t1[:], rstd2_b[:], OP.mult)
                    nc.vector.tensor_tensor(t1[:], t1[:], mr2_b[:], OP.subtract)
                    nc.vector.tensor_scalar(ynT[:, dc, :], t1[:],
                                            lg_c[:, dc:dc + 1], lb_c[:, dc:dc + 1],
                                            OP.mult, OP.add)
                # FFN
                p_o = [psum_po.tile([128, H], F32, name="po", tag="po") for _ in range(DC)]
                for kc in range(KC):
                    p_ff = psum_wk.tile([128, H], F32, tag="work")
                    for kd in range(DC):
                        nc.tensor.matmul(p_ff[:], w1_t[:, kd, kc * 128:(kc + 1) * 128],
                                         ynT[:, kd, :], start=(kd == 0), stop=(kd == DC - 1))
                    ffg = ffgp.tile([128, H], BF16, tag="ffg")
                    nc.scalar.activation(ffg[:], p_ff[:], gelu_func,
                                         bias=b1_c[:, kc:kc + 1])
                    for m2 in range(DC):
                        nc.tensor.matmul(p_o[m2][:], w2_t[:, kc, m2 * 128:(m2 + 1) * 128],
                                         ffg[:], start=(kc == 0), stop=(kc == KC - 1))
                # epilogue: acc += wk * (base + o2 + b2)
                for dc in range(DC):
                    t = slotp.tile([128, H], BF16, tag="epi_tmp")
                    nc.vector.scalar_tensor_tensor(t[:], p_o[dc][:], b2_c[:, dc:dc + 1],
                                                   base[:, dc, :], OP.add, OP.add)
                    if k == 0:
                        nc.vector.tensor_scalar(out_acc[:, dc, hs], t[:],
                                                wk_col, None, OP.mult)
                    else:
                        nc.vector.scalar_tensor_tensor(out_acc[:, dc, hs], t[:],
                                                       wk_col, out_acc[:, dc, hs],
                                                       OP.mult, OP.add)

        def finalize(b):
            d = st[b]
            out_acc = d["out_acc"]
            msq_col = d["msq_col"]
            # out = mask^2 * (x + acc), transposed back to natural
            for sc in range(SC):
                x_sc = xin.tile([128, D], F32, tag="x_fin")
                nc.sync.dma_start(x_sc[:], x_ext[b * S + sc * 128:b * S + (sc + 1) * 128, :])
                onat = onat_p.tile([128, D], F32, tag="onat")
                for dc in range(DC):
                    pt = psum_wk.tile([128, 128], BF16, tag="work")
                    nc.tensor.transpose(pt[:], out_acc[:, dc, sc * 128:(sc + 1) * 128],
                                        ident_bf[:])
                    nc.vector.tensor_copy(onat[:, dc * 128:(dc + 1) * 128], pt[:])
                nc.vector.tensor_tensor(onat[:], onat[:], x_sc[:], OP.add)
                nc.vector.tensor_scalar(onat[:], onat[:], msq_col[:, sc:sc + 1], None, OP.mult)
                nc.sync.dma_start(out_ext[b * S + sc * 128:b * S + (sc + 1) * 128, :],
                                  onat[:])

        # ---------------- schedule ----------------
        phase_a(0); gate(0)
        phase_a(1); gate(1)
        slot(0, 0); slot(0, 1)
        phase_a(2); gate(2)
        finalize(0)
        slot(1, 0); slot(1, 1)
        phase_a(3); gate(3)
        finalize(1)
        slot(2, 0); slot(2, 1)
        finalize(2)
        slot(3, 0); slot(3, 1)
        finalize(3)

        for c in reversed(ctxs):
            c.__exit__(None, None, None)

    nc.finalize()
    return nc


_NC_CACHE = {}


def _get_nc(gelu_func=AF.Gelu):
    key = str(gelu_func)
    if key not in _NC_CACHE:
        _NC_CACHE[key] = build_program(gelu_func)
    return _NC_CACHE[key]


def make_in_maps(inputs):
    x = np.ascontiguousarray(np.asarray(inputs["x"], dtype=np.float32))
    mask = np.ascontiguousarray(np.asarray(inputs["mask"], dtype=np.float32))
    shared = {
        "ln_g": np.ascontiguousarray(inputs["ln_g"], dtype=np.float32),
        "ln_b": np.ascontiguousarray(inputs["ln_b"], dtype=np.float32),
        "conv_w": np.ascontiguousarray(inputs["conv_w"], dtype=np.float32),
        "conv_b": np.ascontiguousarray(inputs["conv_b"], dtype=np.float32),
        "w1": np.ascontiguousarray(inputs["w1"], dtype=np.float32),
        "b1": np.ascontiguousarray(inputs["b1"], dtype=np.float32),
        "w2": np.ascontiguousarray(inputs["w2"], dtype=np.float32),
        "b2": np.ascontiguousarray(inputs["b2"], dtype=np.float32),
        "gln_g": np.ascontiguousarray(np.reshape(np.asarray(inputs["gln_g"], dtype=np.float32), (1, D))),
        "gln_b": np.ascontiguousarray(np.reshape(np.asarray(inputs["gln_b"], dtype=np.float32), (1, D))),
        "gw1": np.ascontiguousarray(inputs["gw1"], dtype=np.float32),
        "gb1": np.ascontiguousarray(np.reshape(np.asarray(inputs["gb1"], dtype=np.float32), (1, D))),
        "gw2": np.ascontiguousarray(inputs["gw2"], dtype=np.float32),
        "gb2": np.ascontiguousarray(np.reshape(np.asarray(inputs["gb2"], dtype=np.float32), (1, E))),
    }
    in_maps = []
    for c in range(NCORES):
        sl = slice(c * NSAMP, (c + 1) * NSAMP)
        m = dict(shared)
        m["x_sh"] = np.ascontiguousarray(x[sl].reshape(NSAMP * S, D))
        m["mask_sh"] = np.ascontiguousarray(mask[sl])
        in_maps.append(m)
    return in_maps


def kernel(**inputs) -> np.ndarray:
    nc = _get_nc()
    in_maps = make_in_maps(inputs)
    res = run_bass_kernel_spmd(nc, in_maps, list(range(NCORES)))
    out = np.concatenate(
        [np.asarray(res.results[c]["out"]).reshape(NSAMP, S, D) for c in range(NCORES)],
        axis=0,
    )
    return out.astype(np.float32)


# revision 12
# speedup vs baseline: 25.3742x; 25.3742x over previous
"""MoEMixer Trainium2 Bass kernel (nn_MoEMixer_66949950210414).

Data-parallel over batch across 8 NeuronCores (4 samples/core).
Per sample: gate (masked mean-pool -> LN -> MLP -> top-2 softmax) in fp32;
per selected expert, weights are fetched from the stacked [E, ...] DRAM
tensors via register-offset dynamic DMA (cast to bf16) and the expert block
runs in transposed-activation layout [D, S]:
  hn    = LN(x)*lg+lb          (stats via natural-layout DVE reduces,
                                applied transposed with PE row-broadcasts)
  convT = depthwise conv in S  (5 PSUM-accumulating matmuls with diagonal
                                bf16 weight matrices)
  base  = convT + cb ; y = x + base
  yn    = LN(y)*lg+lb          (stats via ones-vector bf16 matmuls)
  ffT   = gelu(W1^T yn + b1); o2T = W2^T ffT + b2   (bf16 matmuls; stacked
          weights are the stationary operand -> no weight transposes)
  acc  += wk * (base + o2T)
Because the top-2 weights sum to 1, out = mask^2 * (x + acc); the exact fp32
x residual and the mask are applied at finalize in natural layout after the
PE transpose back.
"""
import numpy as np

import concourse.bacc as bacc
import concourse.tile as tile
from concourse import mybir
from concourse.bass import ds
from concourse.bass_utils import run_bass_kernel_spmd

F32 = mybir.dt.float32
BF16 = mybir.dt.bfloat16
U32 = mybir.dt.uint32
AF = mybir.ActivationFunctionType
OP = mybir.AluOpType
AX = mybir.AxisListType

B, S, D, E, DFF, KS = 32, 1024, 512, 8, 2048, 5
NCORES = 8
NSAMP = B // NCORES          # 4
SC = S // 128                # 8
DC = D // 128                # 4
KC = DFF // 128              # 16
NH = 2
H = S // NH                  # 512
PAD = KS // 2                # 2
EPS = 1e-5


def build_program(gelu_func=AF.Gelu, reps=1):
    nc = bacc.Bacc(None, target_bir_lowering=False)

    x_ext = nc.declare_dram_parameter("x_sh", [NSAMP * S, D], F32, isOutput=False)
    mask_ext = nc.declare_dram_parameter("mask_sh", [NSAMP, S], F32, isOutput=False)
    ln_g_ext = nc.declare_dram_parameter("ln_g", [E, D], F32, isOutput=False)
    ln_b_ext = nc.declare_dram_parameter("ln_b", [E, D], F32, isOutput=False)
    cw_ext = nc.declare_dram_parameter("conv_w", [E, D, KS], F32, isOutput=False)
    cb_ext = nc.declare_dram_parameter("conv_b", [E, D], F32, isOutput=False)
    w1_ext = nc.declare_dram_parameter("w1", [E, D, DFF], F32, isOutput=False)
    b1_ext = nc.declare_dram_parameter("b1", [E, DFF], F32, isOutput=False)
    w2_ext = nc.declare_dram_parameter("w2", [E, DFF, D], F32, isOutput=False)
    b2_ext = nc.declare_dram_parameter("b2", [E, D], F32, isOutput=False)
    gln_g_ext = nc.declare_dram_parameter("gln_g", [1, D], F32, isOutput=False)
    gln_b_ext = nc.declare_dram_parameter("gln_b", [1, D], F32, isOutput=False)
    gw1_ext = nc.declare_dram_parameter("gw1", [D, D], F32, isOutput=False)
    gb1_ext = nc.declare_dram_parameter("gb1", [1, D], F32, isOutput=False)
    gw2_ext = nc.declare_dram_parameter("gw2", [D, E], F32, isOutput=False)
    gb2_ext = nc.declare_dram_parameter("gb2", [1, E], F32, isOutput=False)
    out_ext = nc.declare_dram_parameter("out", [NSAMP * S, D], F32, isOutput=True)

    with tile.TileContext(nc) as tc:
        ctxs = []

        def pool(name, bufs, space=None):
            kw = {"space": space} if space else {}
            p = tc.tile_pool(name=name, bufs=bufs, **kw)
            ctxs.append(p)
            return p.__enter__()

        const = pool("const", 1)
        xin = pool("xin", 2)          # streamed natural x chunks
        samp = pool("samp", 2)        # per-sample persistent (xTb, xhatT, mask cols)
        acc_pool = pool("acc", 1)
        w1p = pool("w1p", 1)
        w2p = pool("w2p", 1)
        slotp = pool("slotp", 2)
        ffgp = pool("ffgp", 3)
        rows = pool("rows", 1)
        cols = pool("cols", 4)
        onat_p = pool("onat", 2)
        psum_po = pool("psum_po", 4, "PSUM")
        psum_wk = pool("psum_wk", 2, "PSUM")
        psum_st = pool("psum_st", 2, "PSUM")

        # ---------------- constants ----------------
        ident = const.tile([128, 128], F32)
        from concourse.masks import make_identity
        make_identity(nc, ident[:])
        ident_bf = const.tile([128, 128], BF16)
        nc.vector.tensor_copy(ident_bf[:], ident[:])

        ones_col_bf = const.tile([128, 1], BF16)
        nc.gpsimd.memset(ones_col_bf[:], 1.0)
        ones_row_bf = const.tile([1, 128], BF16)
        nc.gpsimd.memset(ones_row_bf[:], 1.0)

        gw1_t = const.tile([128, DC, D], F32)
        nc.sync.dma_start(gw1_t[:], gw1_ext[:, :].rearrange("(c p) f -> p c f", p=128))
        gw2_t = const.tile([128, DC, E], F32)
        nc.sync.dma_start(gw2_t[:], gw2_ext[:, :].rearrange("(c p) f -> p c f", p=128))
        gln_g_row = const.tile([1, D], F32)
        nc.sync.dma_start(gln_g_row[:], gln_g_ext[:, :])
        gln_b_row = const.tile([1, D], F32)
        nc.sync.dma_start(gln_b_row[:], gln_b_ext[:, :])
        gb1_row = const.tile([1, D], F32)
        nc.sync.dma_start(gb1_row[:], gb1_ext[:, :])
        gb2_row = const.tile([1, E], F32)
        nc.sync.dma_start(gb2_row[:], gb2_ext[:, :])

        st = [dict() for _ in range(NSAMP)]

        def reset_state():
            for d in st:
                d.clear()

        def bcast(row_bf, h_or_none=None):
            """[1, W] bf16 row -> [128, W] bf16 tile via K=1 matmul."""
            W = row_bf.shape[-1]
            pb = psum_wk.tile([128, W], F32, tag="work", name="pb")
            nc.tensor.matmul(pb[:], ones_row_bf[:], row_bf, start=True, stop=True)
            return pb

        def phase_a(b):
            d = st[b]
            # mask pieces
            mask_row = rows.tile([1, S], F32, tag="mask_row")
            nc.sync.dma_start(mask_row[:], mask_ext[b:b + 1, :])
            mask_cols = samp.tile([128, SC], F32, tag="mask_cols")
            for sc in range(SC):
                nc.sync.dma_start(
                    mask_cols[:, sc:sc + 1],
                    mask_ext[b:b + 1, sc * 128:(sc + 1) * 128].rearrange("o p -> p o"),
                )
            msq_col = samp.tile([128, SC], F32, tag="msq_col")
            nc.vector.tensor_tensor(msq_col[:], mask_cols[:], mask_cols[:], OP.mult)
            d["msq_col"] = msq_col

            xTb = samp.tile([128, DC, S], BF16, tag="xTb")
            sx_c = rows.tile([128, SC], F32, tag="sx_c")
            sq_c = rows.tile([128, SC], F32, tag="sq_c")
            p_gn = psum_st.tile([1, D], F32, tag="stat")
            for sc in range(SC):
                x_sc = xin.tile([128, D], F32, tag="x_sc")
                nc.sync.dma_start(x_sc[:], x_ext[b * S + sc * 128:b * S + (sc + 1) * 128, :])
                # pooled gate input: sum_s x*m
                nc.tensor.matmul(p_gn[:], mask_cols[:, sc:sc + 1], x_sc[:],
                                 start=(sc == 0), stop=(sc == SC - 1))
                # per-position sums for LN1 stats
                nc.vector.tensor_reduce(sx_c[:, sc:sc + 1], x_sc[:], AX.X, OP.add)
                xsq = xin.tile([128, D], BF16, tag="xsq")
                nc.scalar.activation(xsq[:], x_sc[:], AF.Square)
                nc.vector.tensor_reduce(sq_c[:, sc:sc + 1], xsq[:], AX.X, OP.add)
                # transpose x chunk -> bf16 xT
                for dc in range(DC):
                    pt = psum_wk.tile([128, 128], F32, tag="work")
                    nc.tensor.transpose(pt[:], x_sc[:, dc * 128:(dc + 1) * 128], ident[:])
                    nc.vector.tensor_copy(xTb[:, dc, sc * 128:(sc + 1) * 128], pt[:])
            d["xTb"] = xTb

            # gate pooled vector
            dsum = rows.tile([1, 1], F32, tag="dsum")
            nc.vector.tensor_reduce(dsum[:], mask_row[:], AX.X, OP.add)
            nc.vector.tensor_scalar(dsum[:], dsum[:], 1.0, None, OP.max)
            dinv = rows.tile([1, 1], F32, tag="dinv")
            nc.vector.reciprocal(dinv[:], dsum[:])
            g_row = rows.tile([1, D], F32, tag="g_row")
            nc.vector.tensor_scalar(g_row[:], p_gn[:], dinv[0:1, 0:1], None, OP.mult)
            d["g_row"] = g_row

            # LN1 rows: m1, rstd1 per position (row scratch reused in place)
            m1_row = rows.tile([1, S], F32, tag="m1_row")
            q1_row = rows.tile([1, S], F32, tag="q1_row")
            for (src, dst) in ((sx_c, m1_row), (sq_c, q1_row)):
                for sc in range(SC):
                    pt = psum_wk.tile([128, 128], F32, tag="work")
                    nc.tensor.transpose(pt[0:1, :], src[:, sc:sc + 1], ident[:])
                    nc.vector.tensor_scalar(dst[:, sc * 128:(sc + 1) * 128],
                                            pt[0:1, :], 1.0 / D, None, OP.mult)
            v1_row = rows.tile([1, S], F32, tag="v1_row")
            nc.vector.tensor_tensor(v1_row[:], m1_row[:], m1_row[:], OP.mult)
            nc.vector.tensor_tensor(v1_row[:], q1_row[:], v1_row[:], OP.subtract)
            nc.vector.tensor_scalar(v1_row[:], v1_row[:], EPS, None, OP.add)
            nc.scalar.activation(v1_row[:], v1_row[:], AF.Sqrt)
            nc.vector.reciprocal(v1_row[:], v1_row[:])          # = rstd1 (fp32)
            rstd1_bf = rows.tile([1, S], BF16, tag="rstd1_bf")
            nc.vector.tensor_copy(rstd1_bf[:], v1_row[:])
            mr1_bf = rows.tile([1, S], BF16, tag="mr1_bf")
            nc.vector.tensor_tensor(mr1_bf[:], m1_row[:], v1_row[:], OP.mult)

            # xhatT = (x - m1) * rstd1   (bf16; conv input only)
            xhatT = samp.tile([128, DC, S], BF16, tag="xhatT")
            for h in range(NH):
                hs = slice(h * H, (h + 1) * H)
                r_b = slotp.tile([128, H], BF16, tag="r1_b")
                m_b = slotp.tile([128, H], BF16, tag="m1_b")
                for (row, bct) in ((rstd1_bf, r_b), (mr1_bf, m_b)):
                    pb = psum_wk.tile([128, H], F32, tag="work")
                    nc.tensor.matmul(pb[:], ones_row_bf[:], row[:, hs], start=True, stop=True)
                    nc.scalar.activation(bct[:], pb[:], AF.Copy)
                for dc in range(DC):
                    nc.vector.tensor_tensor(xhatT[:, dc, hs], xTb[:, dc, hs], r_b[:], OP.mult)
                    nc.vector.tensor_tensor(xhatT[:, dc, hs], xhatT[:, dc, hs], m_b[:], OP.subtract)
            d["xhatT"] = xhatT

        def gate(b):
            d = st[b]
            g_row = d["g_row"]
            mg = rows.tile([1, 1], F32, tag="mg")
            nc.vector.tensor_reduce(mg[:], g_row[:], AX.X, OP.add)
            nc.vector.tensor_scalar(mg[:], mg[:], 1.0 / D, None, OP.mult)
            gt = rows.tile([1, D], F32, tag="gt")
            nc.vector.tensor_scalar(gt[:], g_row[:], mg[0:1, 0:1], None, OP.subtract)
            gsq = rows.tile([1, D], F32, tag="gsq")
            nc.scalar.activation(gsq[:], gt[:], AF.Square)
            vg = rows.tile([1, 1], F32, tag="vg")
            nc.vector.tensor_reduce(vg[:], gsq[:], AX.X, OP.add)
            nc.vector.tensor_scalar(vg[:], vg[:], 1.0 / D, None, OP.mult)
            nc.vector.tensor_scalar(vg[:], vg[:], EPS, None, OP.add)
            nc.scalar.activation(vg[:], vg[:], AF.Sqrt)
            rg = rows.tile([1, 1], F32, tag="rg")
            nc.vector.reciprocal(rg[:], vg[:])
            h_row = rows.tile([1, D], F32, tag="h_row")
            nc.vector.tensor_scalar(h_row[:], gt[:], rg[0:1, 0:1], None, OP.mult)
            nc.vector.tensor_tensor(h_row[:], h_row[:], gln_g_row[:], OP.mult)
            nc.vector.tensor_tensor(h_row[:], h_row[:], gln_b_row[:], OP.add)
            # h @ gw1 + gb1 -> gelu
            hT = cols.tile([128, DC], F32, tag="hT")
            for kc in range(DC):
                pt = psum_wk.tile([128, 128], F32, tag="work")
                nc.tensor.transpose(pt[:, 0:1], h_row[0:1, kc * 128:(kc + 1) * 128],
                                    ident[0:1, 0:1])
                nc.vector.tensor_copy(hT[:, kc:kc + 1], pt[:, 0:1])
            p_h1 = psum_st.tile([1, D], F32, tag="stat")
            for kc in range(DC):
                nc.tensor.matmul(p_h1[:], hT[:, kc:kc + 1], gw1_t[:, kc, :],
                                 start=(kc == 0), stop=(kc == DC - 1))
            h1_row = rows.tile([1, D], F32, tag="h1_row")
            nc.vector.tensor_tensor(h1_row[:], p_h1[:], gb1_row[:], OP.add)
            hg_row = rows.tile([1, D], F32, tag="hg_row")
            nc.scalar.activation(hg_row[:], h1_row[:], gelu_func)
            # logits
            hgT = cols.tile([128, DC], F32, tag="hgT")
            for kc in range(DC):
                pt = psum_wk.tile([128, 128], F32, tag="work")
                nc.tensor.transpose(pt[:, 0:1], hg_row[0:1, kc * 128:(kc + 1) * 128],
                                    ident[0:1, 0:1])
                nc.vector.tensor_copy(hgT[:, kc:kc + 1], pt[:, 0:1])
            p_lg = psum_st.tile([1, E], F32, tag="stat")
            for kc in range(DC):
                nc.tensor.matmul(p_lg[:], hgT[:, kc:kc + 1], gw2_t[:, kc, :],
                                 start=(kc == 0), stop=(kc == DC - 1))
            logits = rows.tile([1, E], F32, tag="logits")
            nc.vector.tensor_tensor(logits[:], p_lg[:], gb2_row[:], OP.add)
            vals8 = rows.tile([1, 8], F32, tag="vals8")
            idx8 = rows.tile([1, 8], U32, tag="idx8", bufs=2)
            nc.vector.max_with_indices(vals8[:], idx8[:], logits[:])
            d["idx8"] = idx8
            # softmax over top-2
            dd = rows.tile([1, 1], F32, tag="dd")
            nc.vector.tensor_scalar(dd[:], vals8[:, 1:2], vals8[0:1, 0:1], None, OP.subtract)
            ee = rows.tile([1, 1], F32, tag="ee")
            nc.scalar.activation(ee[:], dd[:], AF.Exp)
            nc.vector.tensor_scalar(ee[:], ee[:], 1.0, None, OP.add)
            wA = rows.tile([1, 1], F32, tag="wA")
            nc.vector.reciprocal(wA[:], ee[:])
            wB = rows.tile([1, 1], F32, tag="wB")
            nc.vector.tensor_scalar(wB[:], wA[:], -1.0, 1.0, OP.mult, OP.add)
            wcols = cols.tile([128, 2], F32, tag="wcols")
            nc.gpsimd.partition_broadcast(wcols[:, 0:1], wA[:])
            nc.gpsimd.partition_broadcast(wcols[:, 1:2], wB[:])
            d["wcols"] = wcols

        def slot(b, k):
            d = st[b]
            idx8 = d["idx8"]
            ei_sp = nc.sync.value_load(idx8[0:1, k:k + 1])
            ei_gp = nc.gpsimd.value_load(idx8[0:1, k:k + 1])

            # dynamic weight loads (fp32 -> bf16 cast on SWDGE)
            w1_t = w1p.tile([128, DC, DFF], BF16, tag="w1_t")
            nc.gpsimd.dma_start(
                w1_t[:], w1_ext[ds(ei_gp, 1), :, :].rearrange("o (c p) f -> (o p) c f", p=128))
            w2_t = w2p.tile([128, KC, D], BF16, tag="w2_t")
            nc.gpsimd.dma_start(
                w2_t[:], w2_ext[ds(ei_gp, 1), :, :].rearrange("o (c p) f -> (o p) c f", p=128))
            cw_t = cols.tile([128, DC, KS], F32, tag="cw_t")
            nc.sync.dma_start(
                cw_t[:], cw_ext[ds(ei_sp, 1), :, :].rearrange("o (c p) j -> (o p) c j", p=128))
            lg_c = cols.tile([128, DC], F32, tag="lg_c")
            lb_c = cols.tile([128, DC], F32, tag="lb_c")
            cb_c = cols.tile([128, DC], F32, tag="cb_c")
            b2_c = cols.tile([128, DC], F32, tag="b2_c")
            for dc in range(DC):
                csl = slice(dc * 128, (dc + 1) * 128)
                nc.sync.dma_start(lg_c[:, dc:dc + 1], ln_g_ext[ds(ei_sp, 1), csl].rearrange("o p -> p o"))
                nc.sync.dma_start(lb_c[:, dc:dc + 1], ln_b_ext[ds(ei_sp, 1), csl].rearrange("o p -> p o"))
                nc.sync.dma_start(cb_c[:, dc:dc + 1], cb_ext[ds(ei_sp, 1), csl].rearrange("o p -> p o"))
                nc.sync.dma_start(b2_c[:, dc:dc + 1], b2_ext[ds(ei_sp, 1), csl].rearrange("o p -> p o"))
            b1_c = cols.tile([128, KC], F32, tag="b1_c")
            for kc in range(KC):
                nc.sync.dma_start(
                    b1_c[:, kc:kc + 1],
                    b1_ext[ds(ei_sp, 1), kc * 128:(kc + 1) * 128].rearrange("o p -> p o"))

            xhatT = d["xhatT"]
            xTb = d["xTb"]
            wk_col = d["wcols"][:, k:k + 1]
            if k == 0:
                d["out_acc"] = acc_pool.tile([128, DC, S], BF16, name="out_acc", tag="out_acc")
            out_acc = d["out_acc"]

            for h in range(NH):
                hs = slice(h * H, (h + 1) * H)
                # hn (padded by PAD cols each side, halves overlap via xhatT)
                hn_h = slotp.tile([128, DC, H + 2 * PAD], BF16, tag="hn_h")
                for dc in range(DC):
                    if h == 0:
                        nc.gpsimd.memset(hn_h[:, dc, 0:PAD], 0.0)
                        nc.vector.tensor_scalar(hn_h[:, dc, PAD:H + 2 * PAD],
                                                xhatT[:, dc, 0:H + PAD],
                                                lg_c[:, dc:dc + 1], lb_c[:, dc:dc + 1],
                                                OP.mult, OP.add)
                    else:
                        nc.gpsimd.memset(hn_h[:, dc, H + PAD:H + 2 * PAD], 0.0)
                        nc.vector.tensor_scalar(hn_h[:, dc, 0:H + PAD],
                                                xhatT[:, dc, h * H - PAD:S],
                                                lg_c[:, dc:dc + 1], lb_c[:, dc:dc + 1],
                                                OP.mult, OP.add)
                # conv -> base (conv + cb); y_st = base + x for LN2 stats
                base = slotp.tile([128, DC, H], BF16, tag="base")
                p_sy = psum_st.tile([1, H], F32, tag="stat")
                p_sq = psum_st.tile([1, H], F32, tag="stat")
                for dc in range(DC):
                    diag = slotp.tile([128, KS, 128], BF16, tag="diag")
                    for j in range(KS):
                        nc.vector.tensor_scalar(diag[:, j, :], ident_bf[:],
                                                cw_t[:, dc, j:j + 1], None, OP.mult)
                    p_y = psum_wk.tile([128, H], F32, tag="work")
                    for j in range(KS):
                        nc.tensor.matmul(p_y[:], diag[:, j, :], hn_h[:, dc, j:j + H],
                                         start=(j == 0), stop=(j == KS - 1))
                    nc.vector.tensor_scalar(base[:, dc, :], p_y[:],
                                            cb_c[:, dc:dc + 1], None, OP.add)
                    y_st = slotp.tile([128, H], BF16, tag="y_st")
                    nc.vector.tensor_tensor(y_st[:], base[:, dc, :], xTb[:, dc, hs], OP.add)
                    ysq = slotp.tile([128, H], BF16, tag="ysq")
                    nc.scalar.activation(ysq[:], y_st[:], AF.Square)
                    nc.tensor.matmul(p_sy[:], ones_col_bf[:], y_st[:],
                                     start=(dc == 0), stop=(dc == DC - 1))
                    nc.tensor.matmul(p_sq[:], ones_col_bf[:], ysq[:],
                                     start=(dc == 0), stop=(dc == DC - 1))
                # LN2 rows (scratch reused in place)
                m2_row = rows.tile([1, H], F32, tag="m2_row")
                nc.vector.tensor_scalar(m2_row[:], p_sy[:], 1.0 / D, None, OP.mult)
                v2_row = rows.tile([1, H], F32, tag="v2_row")
                nc.vector.tensor_tensor(v2_row[:], m2_row[:], m2_row[:], OP.mult)
                q2_row = rows.tile([1, H], F32, tag="q2_row")
                nc.vector.tensor_scalar(q2_row[:], p_sq[:], 1.0 / D, None, OP.mult)
                nc.vector.tensor_tensor(v2_row[:], q2_row[:], v2_row[:], OP.subtract)
                nc.vector.tensor_scalar(v2_row[:], v2_row[:], EPS, None, OP.add)
                nc.scalar.activation(v2_row[:], v2_row[:], AF.Sqrt)
                nc.vector.reciprocal(v2_row[:], v2_row[:])       # = rstd2
                rstd2_bf = rows.tile([1, H], BF16, tag="rstd2_bf")
                nc.vector.tensor_copy(rstd2_bf[:], v2_row[:])
                mr2_bf = rows.tile([1, H], BF16, tag="mr2_bf")
                nc.vector.tensor_tensor(mr2_bf[:], m2_row[:], v2_row[:], OP.mult)
                rstd2_b = slotp.tile([128, H], BF16, tag="rstd2_b")
                mr2_b = slotp.tile([128, H], BF16, tag="mr2_b")
                for (row, bct) in ((rstd2_bf, rstd2_b), (mr2_bf, mr2_b)):
                    pb = psum_wk.tile([128, H], F32, tag="work")
                    nc.tensor.matmul(pb[:], ones_row_bf[:], row[:], start=True, stop=True)
                    nc.scalar.activation(bct[:], pb[:], AF.Copy)
                # ynT (bf16)
                ynT = slotp.tile([128, DC, H], BF16, tag="ynT")
                for dc in range(DC):
                    t1 = slotp.tile([128, H], BF16, tag="y_st")
                    nc.vector.tensor_tensor(t1[:], base[:, dc, :], xTb[:, dc, hs], OP.add)
                    nc.vector.tensor_tensor(t1[:], t1[:], rstd2_b[:], OP.mult)
                    nc.vector.tensor_tensor(t1[:], t1[:], mr2_b[:], OP.subtract)
                    nc.vector.tensor_scalar(ynT[:, dc, :], t1[:],
                                            lg_c[:, dc:dc + 1], lb_c[:, dc:dc + 1],
                                            OP.mult, OP.add)
                # FFN
                p_o = [psum_po.tile([128, H], F32, name="po", tag="po") for _ in range(DC)]
                for kc in range(KC):
                    p_ff = psum_wk.tile([128, H], F32, tag="work")
                    for kd in range(DC):
                        nc.tensor.matmul(p_ff[:], w1_t[:, kd, kc * 128:(kc + 1) * 128],
                                         ynT[:, kd, :], start=(kd == 0), stop=(kd == DC - 1))
                    ffg = ffgp.tile([128, H], BF16, tag="ffg")
                    nc.scalar.activation(ffg[:], p_ff[:], gelu_func,
                                         bias=b1_c[:, kc:kc + 1])
                    for m2 in range(DC):
                        nc.tensor.matmul(p_o[m2][:], w2_t[:, kc, m2 * 128:(m2 + 1) * 128],
                                         ffg[:], start=(kc == 0), stop=(kc == KC - 1))
                # epilogue: acc += wk * (base + o2 + b2)
                for dc in range(DC):
                    t = slotp.tile([128, H], BF16, tag="epi_tmp")
                    nc.vector.scalar_tensor_tensor(t[:], p_o[dc][:], b2_c[:, dc:dc + 1],
                                                   base[:, dc, :], OP.add, OP.add)
                    if k == 0:
                        nc.vector.tensor_scalar(out_acc[:, dc, hs], t[:],
                                                wk_col, None, OP.mult)
                    else:
                        nc.vector.scalar_tensor_tensor(out_acc[:, dc, hs], t[:],
                                                       wk_col, out_acc[:, dc, hs],
                                                       OP.mult, OP.add)

        def finalize(b):
            d = st[b]
            out_acc = d["out_acc"]
            msq_col = d["msq_col"]
            # out = mask^2 * (x + acc), transposed back to natural
            for sc in range(SC):
                x_sc = xin.tile([128, D], F32, tag="x_fin")
                nc.sync.dma_start(x_sc[:], x_ext[b * S + sc * 128:b * S + (sc + 1) * 128, :])
                onat = onat_p.tile([128, D], F32, tag="onat")
                for dc in range(DC):
                    pt = psum_wk.tile([128, 128], BF16, tag="work")
                    nc.tensor.transpose(pt[:], out_acc[:, dc, sc * 128:(sc + 1) * 128],
                                        ident_bf[:])
                    nc.vector.tensor_copy(onat[:, dc * 128:(dc + 1) * 128], pt[:])
                nc.vector.tensor_tensor(onat[:], onat[:], x_sc[:], OP.add)
                nc.vector.tensor_scalar(onat[:], onat[:], msq_col[:, sc:sc + 1], None, OP.mult)
                nc.sync.dma_start(out_ext[b * S + sc * 128:b * S + (sc + 1) * 128, :],
                                  onat[:])

        # ---------------- schedule ----------------
        for _rep in range(reps):
            reset_state()
            phase_a(0); gate(0)
            phase_a(1); gate(1)
            slot(0, 0); slot(0, 1)
            phase_a(2); gate(2)
            finalize(0)
            slot(1, 0); slot(1, 1)
            phase_a(3); gate(3)
            finalize(1)
            slot(2, 0); slot(2, 1)
            finalize(2)
            slot(3, 0); slot(3, 1)
            finalize(3)

        for c in reversed(ctxs):
            c.__exit__(None, None, None)

    nc.finalize()
    return nc


_NC_CACHE = {}


def _get_nc(gelu_func=AF.Gelu):
    key = str(gelu_func)
    if key not in _NC_CACHE:
        _NC_CACHE[key] = build_program(gelu_func)
    return _NC_CACHE[key]


def make_in_maps(inputs):
    x = np.ascontiguousarray(np.asarray(inputs["x"], dtype=np.float32))
    mask = np.ascontiguousarray(np.asarray(inputs["mask"], dtype=np.float32))
    shared = {
        "ln_g": np.ascontiguousarray(inputs["ln_g"], dtype=np.float32),
        "ln_b": np.ascontiguousarray(inputs["ln_b"], dtype=np.float32),
        "conv_w": np.ascontiguousarray(inputs["conv_w"], dtype=np.float32),
        "conv_b": np.ascontiguousarray(inputs["conv_b"], dtype=np.float32),
        "w1": np.ascontiguousarray(inputs["w1"], dtype=np.float32),
        "b1": np.ascontiguousarray(inputs["b1"], dtype=np.float32),
        "w2": np.ascontiguousarray(inputs["w2"], dtype=np.float32),
        "b2": np.ascontiguousarray(inputs["b2"], dtype=np.float32),
        "gln_g": np.ascontiguousarray(np.reshape(np.asarray(inputs["gln_g"], dtype=np.float32), (1, D))),
        "gln_b": np.ascontiguousarray(np.reshape(np.asarray(inputs["gln_b"], dtype=np.float32), (1, D))),
        "gw1": np.ascontiguousarray(inputs["gw1"], dtype=np.float32),
        "gb1": np.ascontiguousarray(np.reshape(np.asarray(inputs["gb1"], dtype=np.float32), (1, D))),
        "gw2": np.ascontiguousarray(inputs["gw2"], dtype=np.float32),
        "gb2": np.ascontiguousarray(np.reshape(np.asarray(inputs["gb2"], dtype=np.float32), (1, E))),
    }
    in_maps = []
    for c in range(NCORES):
        sl = slice(c * NSAMP, (c + 1) * NSAMP)
        m = dict(shared)
        m["x_sh"] = np.ascontiguousarray(x[sl].reshape(NSAMP * S, D))
        m["mask_sh"] = np.ascontiguousarray(mask[sl])
        in_maps.append(m)
    return in_maps


def kernel(**inputs) -> np.ndarray:
    nc = _get_nc()
    in_maps = make_in_maps(inputs)
    res = run_bass_kernel_spmd(nc, in_maps, list(range(NCORES)))
    out = np.concatenate(
        [np.asarray(res.results[c]["out"]).reshape(NSAMP, S, D) for c in range(NCORES)],
        axis=0,
    )
    return out.astype(np.float32)


# revision 14
# speedup vs baseline: 27.4721x; 1.0827x over previous
"""MoEMixer Trainium2 Bass kernel (nn_MoEMixer_66949950210414).

Data-parallel over batch across 8 NeuronCores (4 samples/core).
Per sample: gate (masked mean-pool -> LN -> MLP -> top-2 softmax) in fp32;
per selected expert, weights are fetched from the stacked [E, ...] DRAM
tensors via register-offset dynamic DMA (cast to bf16) and the expert block
runs in transposed-activation layout [D, S]:
  hn    = LN(x)*lg+lb          (stats via natural-layout DVE reduces,
                                applied transposed with PE row-broadcasts)
  convT = depthwise conv in S  (5 PSUM-accumulating matmuls with diagonal
                                bf16 weight matrices)
  base  = convT + cb ; y = x + base
  yn    = LN(y)*lg+lb          (stats via ones-vector bf16 matmuls)
  ffT   = gelu(W1^T yn + b1); o2T = W2^T ffT + b2   (bf16 matmuls; stacked
          weights are the stationary operand -> no weight transposes)
  acc  += wk * (base + o2T)
Because the top-2 weights sum to 1, out = mask^2 * (x + acc); the exact fp32
x residual and the mask are applied at finalize in natural layout after the
PE transpose back.
"""
import numpy as np

import concourse.bacc as bacc
import concourse.tile as tile
from concourse import mybir
from concourse.bass import ds
from concourse.bass_utils import run_bass_kernel_spmd

F32 = mybir.dt.float32
BF16 = mybir.dt.bfloat16
U32 = mybir.dt.uint32
AF = mybir.ActivationFunctionType
OP = mybir.AluOpType
AX = mybir.AxisListType

B, S, D, E, DFF, KS = 32, 1024, 512, 8, 2048, 5
NCORES = 8
NSAMP = B // NCORES          # 4
SC = S // 128                # 8
DC = D // 128                # 4
KC = DFF // 128              # 16
NH = 2
H = S // NH                  # 512
PAD = KS // 2                # 2
EPS = 1e-5


def build_program(gelu_func=AF.Gelu, reps=1):
    nc = bacc.Bacc(None, target_bir_lowering=False)

    x_ext = nc.declare_dram_parameter("x_sh", [NSAMP * S, D], F32, isOutput=False)
    mask_ext = nc.declare_dram_parameter("mask_sh", [NSAMP, S], F32, isOutput=False)
    mcols_ext = nc.declare_dram_parameter("mask_cols_sh", [NSAMP, 128, SC], F32, isOutput=False)
    # packed per-expert small params: cols 0-3 lg, 4-7 lb, 8-11 cb, 12-15 b2,
    # 16-31 b1, 32-51 cw (col 32 + dc*KS + j)
    NPC = 4 * DC + KC + DC * KS
    pp_ext = nc.declare_dram_parameter("pp", [E, 128, NPC], F32, isOutput=False)
    w1_ext = nc.declare_dram_parameter("w1_bf", [E, D, DFF], BF16, isOutput=False)
    w2_ext = nc.declare_dram_parameter("w2_bf", [E, DFF, D], BF16, isOutput=False)
    gln_g_ext = nc.declare_dram_parameter("gln_g", [1, D], F32, isOutput=False)
    gln_b_ext = nc.declare_dram_parameter("gln_b", [1, D], F32, isOutput=False)
    gw1_ext = nc.declare_dram_parameter("gw1", [D, D], F32, isOutput=False)
    gb1_ext = nc.declare_dram_parameter("gb1", [1, D], F32, isOutput=False)
    gw2_ext = nc.declare_dram_parameter("gw2", [D, E], F32, isOutput=False)
    gb2_ext = nc.declare_dram_parameter("gb2", [1, E], F32, isOutput=False)
    out_ext = nc.declare_dram_parameter("out", [NSAMP * S, D], F32, isOutput=True)

    with tile.TileContext(nc) as tc:
        ctxs = []

        def pool(name, bufs, space=None):
            kw = {"space": space} if space else {}
            p = tc.tile_pool(name=name, bufs=bufs, **kw)
            ctxs.append(p)
            return p.__enter__()

        const = pool("const", 1)
        xin = pool("xin", 2)          # streamed natural x chunks
        samp = pool("samp", 2)        # per-sample persistent (xTb, xhatT, mask cols)
        acc_pool = pool("acc", 1)
        w1p = pool("w1p", 1)
        w2p = pool("w2p", 1)
        slotp = pool("slotp", 2)
        ffgp = pool("ffgp", 3)
        rows = pool("rows", 1)
        cols = pool("cols", 4)
        onat_p = pool("onat", 2)
        psum_po = pool("psum_po", 4, "PSUM")
        psum_wk = pool("psum_wk", 2, "PSUM")
        psum_st = psum_po

        # ---------------- constants ----------------
        ident = const.tile([128, 128], F32)
        from concourse.masks import make_identity
        make_identity(nc, ident[:])
        ident_bf = const.tile([128, 128], BF16)
        nc.vector.tensor_copy(ident_bf[:], ident[:])

        eps_col = const.tile([128, 1], F32)
        nc.gpsimd.memset(eps_col[:], EPS)
        ones_col_bf = const.tile([128, 1], BF16)
        nc.gpsimd.memset(ones_col_bf[:], 1.0)
        ones_row_bf = const.tile([1, 128], BF16)
        nc.gpsimd.memset(ones_row_bf[:], 1.0)

        gw1_t = const.tile([128, DC, D], F32)
        nc.sync.dma_start(gw1_t[:], gw1_ext[:, :].rearrange("(c p) f -> p c f", p=128))
        gw2_t = const.tile([128, DC, E], F32)
        nc.sync.dma_start(gw2_t[:], gw2_ext[:, :].rearrange("(c p) f -> p c f", p=128))
        gln_g_row = const.tile([1, D], F32)
        nc.sync.dma_start(gln_g_row[:], gln_g_ext[:, :])
        gln_b_row = const.tile([1, D], F32)
        nc.sync.dma_start(gln_b_row[:], gln_b_ext[:, :])
        gb1_row = const.tile([1, D], F32)
        nc.sync.dma_start(gb1_row[:], gb1_ext[:, :])
        gb2_row = const.tile([1, E], F32)
        nc.sync.dma_start(gb2_row[:], gb2_ext[:, :])

        st = [dict() for _ in range(NSAMP)]

        def reset_state():
            for d in st:
                d.clear()

        def bcast(row_bf, h_or_none=None):
            """[1, W] bf16 row -> [128, W] bf16 tile via K=1 matmul."""
            W = row_bf.shape[-1]
            pb = psum_wk.tile([128, W], F32, tag="work", name="pb")
            nc.tensor.matmul(pb[:], ones_row_bf[:], row_bf, start=True, stop=True)
            return pb

        def phase_a(b):
            d = st[b]
            # mask pieces
            mask_row = rows.tile([1, S], F32, tag="mask_row")
            nc.sync.dma_start(mask_row[:], mask_ext[b:b + 1, :])
            mask_cols = samp.tile([128, SC], F32, tag="mask_cols")
            nc.sync.dma_start(mask_cols[:], mcols_ext[b, :, :])
            msq_col = samp.tile([128, SC], F32, tag="msq_col")
            nc.vector.tensor_tensor(msq_col[:], mask_cols[:], mask_cols[:], OP.mult)
            d["msq_col"] = msq_col

            xTb = samp.tile([128, DC, S], BF16, tag="xTb")
            sx_c = rows.tile([128, SC], F32, tag="sx_c")
            sq_c = rows.tile([128, SC], F32, tag="sq_c")
            p_gn = psum_st.tile([1, D], F32, tag="stat", bufs=2)
            for sc in range(SC):
                x_sc = xin.tile([128, D], F32, tag="x_sc")
                nc.sync.dma_start(x_sc[:], x_ext[b * S + sc * 128:b * S + (sc + 1) * 128, :])
                # pooled gate input: sum_s x*m
                nc.tensor.matmul(p_gn[:], mask_cols[:, sc:sc + 1], x_sc[:],
                                 start=(sc == 0), stop=(sc == SC - 1))
                # per-position sums for LN1 stats
                nc.vector.tensor_reduce(sx_c[:, sc:sc + 1], x_sc[:], AX.X, OP.add)
                xsq = xin.tile([128, D], BF16, tag="xsq")
                nc.scalar.activation(xsq[:], x_sc[:], AF.Square)
                nc.vector.tensor_reduce(sq_c[:, sc:sc + 1], xsq[:], AX.X, OP.add)
                # transpose x chunk -> bf16 xT
                for dc in range(DC):
                    pt = psum_wk.tile([128, 128], F32, tag="work")
                    nc.tensor.transpose(pt[:], x_sc[:, dc * 128:(dc + 1) * 128], ident[:])
                    nc.vector.tensor_copy(xTb[:, dc, sc * 128:(sc + 1) * 128], pt[:])
            d["xTb"] = xTb

            # gate pooled vector
            dsum = rows.tile([1, 1], F32, tag="dsum")
            nc.vector.tensor_reduce(dsum[:], mask_row[:], AX.X, OP.add)
            nc.vector.tensor_scalar(dsum[:], dsum[:], 1.0, None, OP.max)
            dinv = rows.tile([1, 1], F32, tag="dinv")
            nc.vector.reciprocal(dinv[:], dsum[:])
            g_row = rows.tile([1, D], F32, tag="g_row")
            nc.vector.tensor_scalar(g_row[:], p_gn[:], dinv[0:1, 0:1], None, OP.mult)
            d["g_row"] = g_row

            # LN1 rows: m1, rstd1 per position (row scratch reused in place)
            m1_row = rows.tile([1, S], F32, tag="m1_row")
            q1_row = rows.tile([1, S], F32, tag="q1_row")
            for (src, dst) in ((sx_c, m1_row), (sq_c, q1_row)):
                for sc in range(SC):
                    pt = psum_wk.tile([128, 128], F32, tag="work")
                    nc.tensor.transpose(pt[0:1, :], src[:, sc:sc + 1], ident[:])
                    nc.vector.tensor_scalar(dst[:, sc * 128:(sc + 1) * 128],
                                            pt[0:1, :], 1.0 / D, None, OP.mult)
            v1_row = rows.tile([1, S], F32, tag="v1_row")
            nc.vector.tensor_tensor(v1_row[:], m1_row[:], m1_row[:], OP.mult)
            nc.vector.tensor_tensor(v1_row[:], q1_row[:], v1_row[:], OP.subtract)
            nc.scalar.activation(v1_row[:], v1_row[:], AF.Sqrt, bias=eps_col[0:1, 0:1])
            nc.vector.reciprocal(v1_row[:], v1_row[:])          # = rstd1 (fp32)
            rstd1_bf = rows.tile([1, S], BF16, tag="rstd1_bf")
            nc.vector.tensor_copy(rstd1_bf[:], v1_row[:])
            mr1_bf = rows.tile([1, S], BF16, tag="mr1_bf")
            nc.vector.tensor_tensor(mr1_bf[:], m1_row[:], v1_row[:], OP.mult)

            # xhatT = (x - m1) * rstd1   (bf16; conv input only)
            xhatT = samp.tile([128, DC, S], BF16, tag="xhatT")
            for h in range(NH):
                hs = slice(h * H, (h + 1) * H)
                r_b = slotp.tile([128, H], BF16, tag="r1_b")
                m_b = slotp.tile([128, H], BF16, tag="m1_b")
                for (row, bct) in ((rstd1_bf, r_b), (mr1_bf, m_b)):
                    pb = psum_wk.tile([128, H], F32, tag="work")
                    nc.tensor.matmul(pb[:], ones_row_bf[:], row[:, hs], start=True, stop=True)
                    nc.scalar.activation(bct[:], pb[:], AF.Copy)
                for dc in range(DC):
                    nc.vector.tensor_tensor(xhatT[:, dc, hs], xTb[:, dc, hs], r_b[:], OP.mult)
                    nc.vector.tensor_tensor(xhatT[:, dc, hs], xhatT[:, dc, hs], m_b[:], OP.subtract)
            d["xhatT"] = xhatT

        def gate(b):
            d = st[b]
            g_row = d["g_row"]
            mg = rows.tile([1, 1], F32, tag="mg")
            nc.vector.tensor_reduce(mg[:], g_row[:], AX.X, OP.add)
            nc.vector.tensor_scalar(mg[:], mg[:], 1.0 / D, None, OP.mult)
            gt = rows.tile([1, D], F32, tag="gt")
            nc.vector.tensor_scalar(gt[:], g_row[:], mg[0:1, 0:1], None, OP.subtract)
            gsq = rows.tile([1, D], F32, tag="gsq")
            nc.scalar.activation(gsq[:], gt[:], AF.Square)
            vg = rows.tile([1, 1], F32, tag="vg")
            nc.vector.tensor_reduce(vg[:], gsq[:], AX.X, OP.add)
            nc.vector.tensor_scalar(vg[:], vg[:], 1.0 / D, None, OP.mult)
            nc.vector.tensor_scalar(vg[:], vg[:], EPS, None, OP.add)
            nc.scalar.activation(vg[:], vg[:], AF.Sqrt)
            rg = rows.tile([1, 1], F32, tag="rg")
            nc.vector.reciprocal(rg[:], vg[:])
            h_row = rows.tile([1, D], F32, tag="h_row")
            nc.vector.tensor_scalar(h_row[:], gt[:], rg[0:1, 0:1], None, OP.mult)
            nc.vector.tensor_tensor(h_row[:], h_row[:], gln_g_row[:], OP.mult)
            nc.vector.tensor_tensor(h_row[:], h_row[:], gln_b_row[:], OP.add)
            # h @ gw1 + gb1 -> gelu
            hT = cols.tile([128, DC], F32, tag="hT")
            for kc in range(DC):
                pt = psum_wk.tile([128, 128], F32, tag="work")
                nc.tensor.transpose(pt[:, 0:1], h_row[0:1, kc * 128:(kc + 1) * 128],
                                    ident[0:1, 0:1])
                nc.vector.tensor_copy(hT[:, kc:kc + 1], pt[:, 0:1])
            p_h1 = psum_st.tile([1, D], F32, tag="stat", bufs=2)
            for kc in range(DC):
                nc.tensor.matmul(p_h1[:], hT[:, kc:kc + 1], gw1_t[:, kc, :],
                                 start=(kc == 0), stop=(kc == DC - 1))
            h1_row = rows.tile([1, D], F32, tag="h1_row")
            nc.vector.tensor_tensor(h1_row[:], p_h1[:], gb1_row[:], OP.add)
            hg_row = rows.tile([1, D], F32, tag="hg_row")
            nc.scalar.activation(hg_row[:], h1_row[:], gelu_func)
            # logits
            hgT = cols.tile([128, DC], F32, tag="hgT")
            for kc in range(DC):
                pt = psum_wk.tile([128, 128], F32, tag="work")
                nc.tensor.transpose(pt[:, 0:1], hg_row[0:1, kc * 128:(kc + 1) * 128],
                                    ident[0:1, 0:1])
                nc.vector.tensor_copy(hgT[:, kc:kc + 1], pt[:, 0:1])
            p_lg = psum_st.tile([1, E], F32, tag="stat", bufs=2)
            for kc in range(DC):
                nc.tensor.matmul(p_lg[:], hgT[:, kc:kc + 1], gw2_t[:, kc, :],
                                 start=(kc == 0), stop=(kc == DC - 1))
            logits = rows.tile([1, E], F32, tag="logits")
            nc.vector.tensor_tensor(logits[:], p_lg[:], gb2_row[:], OP.add)
            vals8 = rows.tile([1, 8], F32, tag="vals8")
            idx8 = rows.tile([1, 8], U32, tag="idx8", bufs=2)
            nc.vector.max_with_indices(vals8[:], idx8[:], logits[:])
            d["idx8"] = idx8
            # softmax over top-2
            dd = rows.tile([1, 1], F32, tag="dd")
            nc.vector.tensor_scalar(dd[:], vals8[:, 1:2], vals8[0:1, 0:1], None, OP.subtract)
            ee = rows.tile([1, 1], F32, tag="ee")
            nc.scalar.activation(ee[:], dd[:], AF.Exp)
            nc.vector.tensor_scalar(ee[:], ee[:], 1.0, None, OP.add)
            wA = rows.tile([1, 1], F32, tag="wA")
            nc.vector.reciprocal(wA[:], ee[:])
            wB = rows.tile([1, 1], F32, tag="wB")
            nc.vector.tensor_scalar(wB[:], wA[:], -1.0, 1.0, OP.mult, OP.add)
            wcols = cols.tile([128, 2], F32, tag="wcols")
            nc.gpsimd.partition_broadcast(wcols[:, 0:1], wA[:])
            nc.gpsimd.partition_broadcast(wcols[:, 1:2], wB[:])
            d["wcols"] = wcols

        def slot(b, k):
            d = st[b]
            idx8 = d["idx8"]
            ei_sp = nc.sync.value_load(idx8[0:1, k:k + 1])
            ei_ac = nc.scalar.value_load(idx8[0:1, k:k + 1])

            # dynamic weight loads (bf16, plain HWDGE on two rings)
            w1_t = w1p.tile([128, DC, DFF], BF16, tag="w1_t")
            nc.scalar.dma_start(
                w1_t[:], w1_ext[ds(ei_ac, 1), :, :].rearrange("o (c p) f -> (o p) c f", p=128))
            w2_t = w2p.tile([128, KC, D], BF16, tag="w2_t")
            nc.sync.dma_start(
                w2_t[:], w2_ext[ds(ei_sp, 1), :, :].rearrange("o (c p) f -> (o p) c f", p=128))
            NPC = 4 * DC + KC + DC * KS
            pp_t = cols.tile([128, NPC], F32, tag="pp_t")
            nc.sync.dma_start(pp_t[:], pp_ext[ds(ei_sp, 1), :, :].rearrange("o p c -> (o p) c"))
            lg_c = pp_t[:, 0:DC]
            lb_c = pp_t[:, DC:2 * DC]
            cb_c = pp_t[:, 2 * DC:3 * DC]
            b2_c = pp_t[:, 3 * DC:4 * DC]
            b1_c = pp_t[:, 4 * DC:4 * DC + KC]
            cw_t = pp_t[:, 4 * DC + KC:].rearrange("p (c j) -> p c j", j=KS)

            xhatT = d["xhatT"]
            xTb = d["xTb"]
            wk_col = d["wcols"][:, k:k + 1]
            if k == 0:
                d["out_acc"] = acc_pool.tile([128, DC, S], BF16, name="out_acc", tag="out_acc")
            out_acc = d["out_acc"]

            for h in range(NH):
                hs = slice(h * H, (h + 1) * H)
                # hn (padded by PAD cols each side, halves overlap via xhatT)
                hn_h = slotp.tile([128, DC, H + 2 * PAD], BF16, tag="hn_h")
                for dc in range(DC):
                    if h == 0:
                        nc.gpsimd.memset(hn_h[:, dc, 0:PAD], 0.0)
                        nc.vector.tensor_scalar(hn_h[:, dc, PAD:H + 2 * PAD],
                                                xhatT[:, dc, 0:H + PAD],
                                                lg_c[:, dc:dc + 1], lb_c[:, dc:dc + 1],
                                                OP.mult, OP.add)
                    else:
                        nc.gpsimd.memset(hn_h[:, dc, H + PAD:H + 2 * PAD], 0.0)
                        nc.vector.tensor_scalar(hn_h[:, dc, 0:H + PAD],
                                                xhatT[:, dc, h * H - PAD:S],
                                                lg_c[:, dc:dc + 1], lb_c[:, dc:dc + 1],
                                                OP.mult, OP.add)
                # conv -> base (conv + cb); y_st = base + x for LN2 stats
                base = slotp.tile([128, DC, H], BF16, tag="base")
                p_sy = psum_st.tile([1, H], F32, tag="stat", bufs=2)
                p_sq = psum_st.tile([1, H], F32, tag="stat", bufs=2)
                for dc in range(DC):
                    diag = slotp.tile([128, KS, 128], BF16, tag="diag")
                    for j in range(KS):
                        nc.vector.tensor_scalar(diag[:, j, :], ident_bf[:],
                                                cw_t[:, dc, j:j + 1], None, OP.mult)
                    p_y = psum_wk.tile([128, H], F32, tag="work")
                    for j in range(KS):
                        nc.tensor.matmul(p_y[:], diag[:, j, :], hn_h[:, dc, j:j + H],
                                         start=(j == 0), stop=(j == KS - 1))
                    nc.vector.tensor_scalar(base[:, dc, :], p_y[:],
                                            cb_c[:, dc:dc + 1], None, OP.add)
                    y_st = slotp.tile([128, H], BF16, tag="y_st")
                    nc.vector.tensor_tensor(y_st[:], base[:, dc, :], xTb[:, dc, hs], OP.add)
                    ysq = slotp.tile([128, H], BF16, tag="ysq")
                    nc.scalar.activation(ysq[:], y_st[:], AF.Square)
                    nc.tensor.matmul(p_sy[:], ones_col_bf[:], y_st[:],
                                     start=(dc == 0), stop=(dc == DC - 1))
                    nc.tensor.matmul(p_sq[:], ones_col_bf[:], ysq[:],
                                     start=(dc == 0), stop=(dc == DC - 1))
                # LN2 rows: rstd2 = 1/sqrt(q/D - (sy/D)^2 + eps)
                m2_row = rows.tile([1, H], F32, tag="m2_row")
                nc.vector.tensor_scalar(m2_row[:], p_sy[:], 1.0 / D, None, OP.mult)
                msq2 = rows.tile([1, H], F32, tag="msq2")
                nc.scalar.activation(msq2[:], p_sy[:], AF.Square, scale=1.0 / D)
                v2_row = rows.tile([1, H], F32, tag="v2_row")
                nc.vector.scalar_tensor_tensor(v2_row[:], p_sq[:], 1.0 / D, msq2[:],
                                               OP.mult, OP.subtract)
                nc.scalar.activation(v2_row[:], v2_row[:], AF.Sqrt, bias=eps_col[0:1, 0:1])
                nc.vector.reciprocal(v2_row[:], v2_row[:])       # = rstd2
                rstd2_bf = rows.tile([1, H], BF16, tag="rstd2_bf")
                nc.vector.tensor_copy(rstd2_bf[:], v2_row[:])
                mr2_bf = rows.tile([1, H], BF16, tag="mr2_bf")
                nc.vector.tensor_tensor(mr2_bf[:], m2_row[:], v2_row[:], OP.mult)
                rstd2_b = slotp.tile([128, H], BF16, tag="rstd2_b")
                mr2_b = slotp.tile([128, H], BF16, tag="mr2_b")
                for (row, bct) in ((rstd2_bf, rstd2_b), (mr2_bf, mr2_b)):
                    pb = psum_wk.tile([128, H], F32, tag="work")
                    nc.tensor.matmul(pb[:], ones_row_bf[:], row[:], start=True, stop=True)
                    nc.scalar.activation(bct[:], pb[:], AF.Copy)
                # ynT (bf16)
                ynT = slotp.tile([128, DC, H], BF16, tag="ynT")
                for dc in range(DC):
                    t1 = slotp.tile([128, H], BF16, tag="y_st")
                    nc.vector.tensor_tensor(t1[:], base[:, dc, :], xTb[:, dc, hs], OP.add)
                    nc.vector.tensor_tensor(t1[:], t1[:], rstd2_b[:], OP.mult)
                    nc.vector.tensor_tensor(t1[:], t1[:], mr2_b[:], OP.subtract)
                    nc.vector.tensor_scalar(ynT[:, dc, :], t1[:],
                                            lg_c[:, dc:dc + 1], lb_c[:, dc:dc + 1],
                                            OP.mult, OP.add)
                # FFN
                p_o = [psum_po.tile([128, H], F32, name="po", tag="po") for _ in range(DC)]
                for kc in range(KC):
                    p_ff = psum_wk.tile([128, H], F32, tag="work")
                    for kd in range(DC):
                        nc.tensor.matmul(p_ff[:], w1_t[:, kd, kc * 128:(kc + 1) * 128],
                                         ynT[:, kd, :], start=(kd == 0), stop=(kd == DC - 1))
                    ffg = ffgp.tile([128, H], BF16, tag="ffg")
                    nc.scalar.activation(ffg[:], p_ff[:], gelu_func,
                                         bias=b1_c[:, kc:kc + 1])
                    for m2 in range(DC):
                        nc.tensor.matmul(p_o[m2][:], w2_t[:, kc, m2 * 128:(m2 + 1) * 128],
                                         ffg[:], start=(kc == 0), stop=(kc == KC - 1))
                # epilogue: acc += wk * (base + o2 + b2)
                for dc in range(DC):
                    t = slotp.tile([128, H], BF16, tag="epi_tmp")
                    nc.vector.scalar_tensor_tensor(t[:], p_o[dc][:], b2_c[:, dc:dc + 1],
                                                   base[:, dc, :], OP.add, OP.add)
                    if k == 0:
                        nc.vector.tensor_scalar(out_acc[:, dc, hs], t[:],
                                                wk_col, None, OP.mult)
                    else:
                        nc.vector.scalar_tensor_tensor(out_acc[:, dc, hs], t[:],
                                                       wk_col, out_acc[:, dc, hs],
                                                       OP.mult, OP.add)

        def finalize(b):
            d = st[b]
            out_acc = d["out_acc"]
            msq_col = d["msq_col"]
            # out = mask^2 * (x + acc), transposed back to natural
            for sc in range(SC):
                x_sc = xin.tile([128, D], F32, tag="x_fin")
                nc.sync.dma_start(x_sc[:], x_ext[b * S + sc * 128:b * S + (sc + 1) * 128, :])
                onat = onat_p.tile([128, D], F32, tag="onat")
                for dc in range(DC):
                    pt = psum_wk.tile([128, 128], BF16, tag="work")
                    nc.tensor.transpose(pt[:], out_acc[:, dc, sc * 128:(sc + 1) * 128],
                                        ident_bf[:])
                    nc.vector.tensor_copy(onat[:, dc * 128:(dc + 1) * 128], pt[:])
                nc.vector.tensor_tensor(onat[:], onat[:], x_sc[:], OP.add)
                nc.vector.tensor_scalar(onat[:], onat[:], msq_col[:, sc:sc + 1], None, OP.mult)
                nc.sync.dma_start(out_ext[b * S + sc * 128:b * S + (sc + 1) * 128, :],
                                  onat[:])

        # ---------------- schedule ----------------
        for _rep in range(reps):
            reset_state()
            phase_a(0); gate(0)
            phase_a(1); gate(1)
            slot(0, 0); slot(0, 1)
            phase_a(2); gate(2)
            finalize(0)
            slot(1, 0); slot(1, 1)
            phase_a(3); gate(3)
            finalize(1)
            slot(2, 0); slot(2, 1)
            finalize(2)
            slot(3, 0); slot(3, 1)
            finalize(3)

        for c in reversed(ctxs):
            c.__exit__(None, None, None)

    nc.finalize()
    return nc


_NC_CACHE = {}


def _get_nc(gelu_func=AF.Gelu):
    key = str(gelu_func)
    if key not in _NC_CACHE:
        _NC_CACHE[key] = build_program(gelu_func)
    return _NC_CACHE[key]


def make_in_maps(inputs):
    import ml_dtypes
    x = np.ascontiguousarray(np.asarray(inputs["x"], dtype=np.float32))
    mask = np.ascontiguousarray(np.asarray(inputs["mask"], dtype=np.float32))
    # pack per-expert small params into [E, 128, NPC]
    NPC = 4 * DC + KC + DC * KS
    pp = np.zeros((E, 128, NPC), np.float32)
    ln_g = np.asarray(inputs["ln_g"], dtype=np.float32).reshape(E, DC, 128)
    ln_b = np.asarray(inputs["ln_b"], dtype=np.float32).reshape(E, DC, 128)
    cb = np.asarray(inputs["conv_b"], dtype=np.float32).reshape(E, DC, 128)
    b2 = np.asarray(inputs["b2"], dtype=np.float32).reshape(E, DC, 128)
    b1 = np.asarray(inputs["b1"], dtype=np.float32).reshape(E, KC, 128)
    cw = np.asarray(inputs["conv_w"], dtype=np.float32).reshape(E, DC, 128, KS)
    pp[:, :, 0:DC] = ln_g.transpose(0, 2, 1)
    pp[:, :, DC:2 * DC] = ln_b.transpose(0, 2, 1)
    pp[:, :, 2 * DC:3 * DC] = cb.transpose(0, 2, 1)
    pp[:, :, 3 * DC:4 * DC] = b2.transpose(0, 2, 1)
    pp[:, :, 4 * DC:4 * DC + KC] = b1.transpose(0, 2, 1)
    pp[:, :, 4 * DC + KC:] = cw.transpose(0, 2, 1, 3).reshape(E, 128, DC * KS)
    mask_cols = np.ascontiguousarray(
        mask.reshape(B, S // 128, 128).transpose(0, 2, 1))  # [B, 128, SC]
    shared = {
        "pp": np.ascontiguousarray(pp),
        "w1_bf": np.ascontiguousarray(
            np.asarray(inputs["w1"], dtype=np.float32).astype(ml_dtypes.bfloat16)),
        "w2_bf": np.ascontiguousarray(
            np.asarray(inputs["w2"], dtype=np.float32).astype(ml_dtypes.bfloat16)),
        "gln_g": np.ascontiguousarray(np.reshape(np.asarray(inputs["gln_g"], dtype=np.float32), (1, D))),
        "gln_b": np.ascontiguousarray(np.reshape(np.asarray(inputs["gln_b"], dtype=np.float32), (1, D))),
        "gw1": np.ascontiguousarray(inputs["gw1"], dtype=np.float32),
        "gb1": np.ascontiguousarray(np.reshape(np.asarray(inputs["gb1"], dtype=np.float32), (1, D))),
        "gw2": np.ascontiguousarray(inputs["gw2"], dtype=np.float32),
        "gb2": np.ascontiguousarray(np.reshape(np.asarray(inputs["gb2"], dtype=np.float32), (1, E))),
    }
    in_maps = []
    for c in range(NCORES):
        sl = slice(c * NSAMP, (c + 1) * NSAMP)
        m = dict(shared)
        m["x_sh"] = np.ascontiguousarray(x[sl].reshape(NSAMP * S, D))
        m["mask_sh"] = np.ascontiguousarray(mask[sl])
        m["mask_cols_sh"] = np.ascontiguousarray(mask_cols[sl])
        in_maps.append(m)
    return in_maps


def kernel(**inputs) -> np.ndarray:
    nc = _get_nc()
    in_maps = make_in_maps(inputs)
    res = run_bass_kernel_spmd(nc, in_maps, list(range(NCORES)))
    out = np.concatenate(
        [np.asarray(res.results[c]["out"]).reshape(NSAMP, S, D) for c in range(NCORES)],
        axis=0,
    )
    return out.astype(np.float32)
